# revision 27
# baseline (speedup 1.0000x reference)
"""BiAttention (mode==1) Trainium2 Bass kernel.

Reference computation (per batch b, for (W,bias) in [(W2,b2),(W3,b3)]):
    proj   = input2[b] @ W.T + bias          # [S, D]
    scores = input1[b] @ proj.T              # [T, S]
    w      = softmax(scores, axis=-1)
    out    = w @ input2[b]                   # [T, D]
with B=16, T=2048, S=1024, D=300.

Key restructurings (validated vs reference in fp64/fp32, absmax-rel ~6e-3):
  * The bias contributes sum_e bias[e]*input1[b,t,e] to scores — constant in s,
    so it cancels in softmax and is dropped entirely.
  * Everything is computed in the transposed "scoresT" orientation [s, t] so
    that every matmul contracts over the partition dim with NO on-chip
    transposes:
        projT  [e, s] = Wt.T @ input2T      (lhsT = W.T chunks, rhs = input2T)
        scoresT[s, t] = projT.T @ input1T   (lhsT = projT slices, rhs = input1T)
        E = exp(scoresT)                    (no max-subtraction: |scores| < ~60,
                                             fine in f32 PSUM -> bf16)
        out[t, :304]  = E.T @ [input2 | 1]  (ones column accumulates the
                                             softmax denominator for free)
        out[t, d] = out[t, d] / out[t, 300]
  * proj/input1/W run in float16 (11-bit mantissa: scores err ~3e-3 absolute,
    harmless through exp; bf16 would NOT be - measured 3.9e-2 rel err).  E and
    input2 stay bf16 for mm4 (E spans e^+-60: needs bf16 range).  fp16 also
    halves the input DMA and gets LDWEIGHTS fast-weight-load (f32r cannot).
  * D=300 is chunked 128+128+128(44 real+zeros).  The two attentions share
    every loop; the ragged third chunk of mm1 packs both attentions' 64
    output columns into single 128-col weights ("pk"), producing projT-chunk2
    as [A @ partitions 0-63 | B @ partitions 64-127].  mm2's third chunk then
    uses that packed tile as lhsT for BOTH attentions, with two host-prepared
    rhs variants of input1T-chunk2 (data in the low / high partition half,
    zeros elsewhere) selecting the right half of the contraction.
    All matmuls stay uniform [K=128, M=128, N=512] - measured on HW that
    row-tiled 64-row pairs do NOT stream concurrently for this path and also
    break LDWEIGHTS pull-ahead on the following matmul.
  * PE warm-up: 8 dummy matmuls on a zeroed tile run during the initial DMA
    wait so the HAM clock gate reaches 2.4 GHz before real work (and no
    PE gaps afterwards, so it never re-throttles - worth ~2x on every
    matmul that would otherwise run cold).
  * mm4 (+softmax normalize) for t-block tb is issued AFTER mm2 of tb+1, so
    the scalar engine's exp stream (1.38us/st-group vs PE's ~1.35) never
    gates the PE within an mm2 phase.
  * Head DMAs are need-ordered and split to ~128-256KB so several of the 16
    DMA queues run in parallel (one queue sustains only ~96GB/s); issued on
    the two hardware DGE paths (sync + scalar).  gpsimd's software DGE is
    slow - not used.
  * Data-parallel over batch: 8 cores x 2 batches each, params replicated.
"""

import os

import numpy as np

B, T, S, D = 16, 2048, 1024, 300
NA = 304          # input2 free dim: 300 data + ones col at 300 + pad
                  # (301 fails walrus "ISA check" on the matmul)
NB = 2            # batches per core
NCORES = 8
NT = T // 512     # 4 t-blocks of 512
NS = S // 128     # 8 s-chunks of 128

_CACHE = {}


def _split_multi_waits(nc, maxw=1):
    """This walrus/CoreV3 build accepts at most one semaphore sync-wait per
    instruction ("Too many sync wait commands").  Tile attaches several to
    matmuls/DMAs/the tail Drain.  Post-scheduling, splice NOP carrier
    instructions (one wait each) in front of any instruction with more."""
    import concourse.mybir as mybir

    ctr = 0
    for fn in nc.m.functions:
        for blk in fn.blocks:
            insts = blk.instructions
            i = 0
            while i < len(insts):
                inst = insts[i]
                si = getattr(inst, "sync_info", None)
                waits = list(si.on_wait) if si is not None and si.on_wait else []
                if len(waits) > maxw:
                    si.on_wait = waits[len(waits) - maxw :]
                    carriers = []
                    for w in waits[: len(waits) - maxw]:
                        ctr += 1
                        carriers.append(
                            mybir.InstNoOp(
                                name=f"waitsplit-{ctr}",
                                engine=inst.engine,
                                ins=[],
                                outs=[],
                                sync_info=mybir.SyncInfo(on_wait=[w], on_update=[]),
                                bass_nofuse=True,
                            )
                        )
                    insts[i:i] = carriers
                    i += len(carriers)
                i += 1


def _install_profile_hook():
    """Synthesize the missing ``antenv.axon_hooks`` glue so run_bass_kernel_spmd
    trace=True can drive NTFF profiling through the injected libaxon_pjrt.so,
    and stub out the artifact upload (no bucket access here)."""
    import sys
    import types

    if "antenv.axon_hooks" not in sys.modules:
        mod = types.ModuleType("antenv.axon_hooks")
        holder = {}
        mod.set_axon_ntff_profile_hook = lambda h: holder.__setitem__("h", h)
        mod.get_axon_ntff_profile_hook = lambda: holder.get("h")
        sys.modules["antenv.axon_hooks"] = mod
        try:
            from trn_agent_boot.trn_boot import _ntff_profile_via_ctypes

            mod.set_axon_ntff_profile_hook(
                _ntff_profile_via_ctypes("/opt/axon/libaxon_pjrt.so")
            )
        except Exception:
            pass

    import concourse.bass_utils as bu

    if not getattr(bu, "_upload_stubbed", False):
        bu.upload_artifacts = lambda tmpdir: f"local:{tmpdir}"
        bu._upload_stubbed = True


def _build_nc():
    import concourse.bass as bass
    import concourse.mybir as mybir
    from concourse.tile import TileContext

    f32 = mybir.dt.float32
    f16 = mybir.dt.float16
    bf16 = mybir.dt.bfloat16
    Exp = mybir.ActivationFunctionType.Exp

    nc = bass.Bass("TRN2", target_bir_lowering=False, debug=False)
    # wts j-index (need-ordered for mm1's group sequence):
    #   A-ke0: j0-2 (kd0,1,2)   B-ke0: j3-5   A-ke1: j6-8   B-ke1: j9-11
    #   pk (A|B e2-cols packed): j12-14
    wts = nc.declare_dram_parameter("wts", [128, 15, 128], f16, isOutput=False)
    # in1t c-index: 0,1 = e-chunks 0,1; 2 = chunk2 data @ p0-63 (zeros @ hi);
    # 3 = chunk2 data @ p64-127 (zeros @ lo)
    in1t = nc.declare_dram_parameter("in1t", [NB, 4, 128, T], f16, isOutput=False)
    in2t = nc.declare_dram_parameter("in2t", [NB, 2, 3, 128, 512], f16, isOutput=False)
    in2n = nc.declare_dram_parameter("in2n", [NB, 128, NS, NA], bf16, isOutput=False)
    out_h = [
        nc.declare_dram_parameter("out_a", [NB, NT, 128, 4, D], f32, isOutput=True),
        nc.declare_dram_parameter("out_b", [NB, NT, 128, 4, D], f32, isOutput=True),
    ]

    with TileContext(nc) as tc:
        with (
            tc.tile_pool(name="warmp", bufs=1) as warmp,
            tc.tile_pool(name="wpool", bufs=1) as wpool,
            tc.tile_pool(name="a1p", bufs=2) as a1p,
            tc.tile_pool(name="a2p", bufs=2) as a2p,
            tc.tile_pool(name="a2np", bufs=2) as a2np,
            tc.tile_pool(name="projp", bufs=2) as projp,
            tc.tile_pool(name="ep", bufs=2) as ep,
            tc.tile_pool(name="outp", bufs=3) as outp,
            tc.tile_pool(name="recp", bufs=4) as recp,
            tc.tile_pool(name="ps_pj", bufs=2, space="PSUM") as ps_pj,
            tc.tile_pool(name="ps_sc", bufs=4, space="PSUM") as ps_sc,
            tc.tile_pool(name="ps_o", bufs=2, space="PSUM") as ps_o,
        ):
            # --- PE warm-up + gap fillers.  The head phase is DMA-paced: data
            # trickles in slower than the PE consumes it, and any PE idle
            # keeps the HAM clock gate at 1.2 GHz (everything then runs at
            # half clock until ~3.4us of sustained busy).  Dummy matmuls on an
            # intentionally-uninitialized tile (results never read, no deps,
            # so they slot into exactly the cycles the real stream would
            # stall) keep the busy-streak alive.  13 up front bridge preamble
            # -> first data; small fillers between the DMA-paced mm1 groups
            # bridge the trickle gaps.
            wsb = warmp.tile([128, 512], bf16)
            wscr = warmp.tile([128, 1], f32)
            nc.vector.memset(wsb, 0.0)
            wps = ps_o.tile([128, 512], f32, tag="o")

            def filler(n):
                for _ in range(n):
                    nc.tensor.matmul(
                        wps[:, 0:256], wsb[:, 0:128], wsb[:, 0:256],
                        start=True, stop=True,
                    )

            filler(17)

            # --- weights: resident all kernel, need-ordered DMAs.
            wt = wpool.tile([128, 15, 128], f16)

            for lb in range(NB):
                a2 = a2p.tile([128, 2, 3, 512], f16)
                a1 = a1p.tile([128, 4, T], f16)
                a2n = a2np.tile([128, NS, NA], bf16)
                if lb == 0:
                    # Both hardware DGEs (sync=SP, scalar=ACT) interleave the
                    # mm1 critical path: each dma_start costs ~650ns of issue
                    # time on its engine, so one engine alone delivers too
                    # late.  The dummy activation early on the scalar queue
                    # pre-triggers the ~2.7us exp ACT_TABLE_LOAD, which would
                    # otherwise stall the first real softmax by that much.
                    nc.sync.dma_start(out=wt[:, 0:3, :], in_=wts[:, 0:3, :])
                    nc.scalar.dma_start(out=a2[:, 0, 1], in_=in2t[lb, 0, 1])
                    nc.sync.dma_start(out=a2[:, 0, 0], in_=in2t[lb, 0, 0])
                    nc.scalar.dma_start(out=a2[:, 0, 2], in_=in2t[lb, 0, 2])
                    nc.scalar.activation(out=wscr, in_=wsb[:, 0:1], func=Exp)
                    nc.sync.dma_start(out=wt[:, 3:6, :], in_=wts[:, 3:6, :])
                    nc.scalar.dma_start(out=a2[:, 1, 1], in_=in2t[lb, 1, 1])
                    nc.sync.dma_start(out=a2[:, 1, 0], in_=in2t[lb, 1, 0])
                    nc.scalar.dma_start(out=a2[:, 1, 2], in_=in2t[lb, 1, 2])
                    for c in range(4):
                        nc.sync.dma_start(
                            out=a1[:, c, 0:1024], in_=in1t[lb, c][:, 0:1024]
                        )
                    nc.sync.dma_start(out=wt[:, 6:12, :], in_=wts[:, 6:12, :])
                    nc.sync.dma_start(out=wt[:, 12:15, :], in_=wts[:, 12:15, :])
                    nc.scalar.dma_start(out=a2n, in_=in2n[lb])
                    for c in range(4):
                        nc.sync.dma_start(
                            out=a1[:, c, 1024:T], in_=in1t[lb, c][:, 1024:T]
                        )
                else:
                    for h in range(2):
                        for kd in range(3):
                            nc.sync.dma_start(out=a2[:, h, kd], in_=in2t[lb, h, kd])
                    for c in range(4):
                        nc.sync.dma_start(out=a1[:, c, :], in_=in1t[lb, c])
                    nc.scalar.dma_start(out=a2n, in_=in2n[lb])

                # --- mm1: projT.  5 uniform 3-matmul groups per h:
                # A-ke0, B-ke0, A-ke1, B-ke1, pk (both attns' e2 cols packed).
                ptA = projp.tile([128, 2, S], f16)
                ptB = projp.tile([128, 2, S], f16)
                ptAB2 = projp.tile([128, S], f16)
                Copy = mybir.ActivationFunctionType.Copy
                for h in range(2):
                    hs = slice(h * 512, (h + 1) * 512)
                    for gi, (g, dst) in enumerate((
                        (0, lambda: ptA[:, 0, hs]),
                        (3, lambda: ptB[:, 0, hs]),
                        (6, lambda: ptA[:, 1, hs]),
                        (9, lambda: ptB[:, 1, hs]),
                        (12, lambda: ptAB2[:, hs]),
                    )):
                        pj = ps_pj.tile([128, 512], f32, tag="pj", name=f"pj{lb}{h}{g}")
                        for kd in range(3):
                            nc.tensor.matmul(
                                pj, wt[:, g + kd, :], a2[:, h, kd, :],
                                start=(kd == 0), stop=(kd == 2),
                            )
                        # alternate the PSUM->SBUF drains over two engines so
                        # the 2-bank pj ring never gates the matmul stream
                        if gi % 2 == 1:
                            nc.scalar.activation(out=dst(), in_=pj, func=Copy)
                        else:
                            nc.vector.tensor_copy(dst(), pj)
                        if lb == 0 and h == 0 and gi < 3:
                            filler(2)
                if lb == 0:
                    filler(3)

                # mm4 runs one t-block behind mm2 so the scalar engine's exp
                # stream (slower than the mm2 matmul stream) never gates PE.
                pending = None

                def mm4_block(tb, EA, EB):
                    for a in range(2):
                        E = EA if a == 0 else EB
                        ostg = outp.tile(
                            [128, 4, D], f32, tag="ostg", name=f"ostg{lb}{tb}{a}"
                        )
                        for ts in range(4):
                            o = ps_o.tile(
                                [128, NA], f32, tag="o", name=f"o{lb}{tb}{a}{ts}"
                            )
                            for st in range(NS):
                                nc.tensor.matmul(
                                    o,
                                    E[:, st, ts * 128 : (ts + 1) * 128],
                                    a2n[:, st, :],
                                    start=(st == 0),
                                    stop=(st == NS - 1),
                                )
                            rec = recp.tile(
                                [128, 1], f32, tag="rec", name=f"rec{lb}{tb}{a}{ts}"
                            )
                            nc.vector.reciprocal(rec, o[:, 300:301])
                            final = lb == NB - 1 and a == 1 and tb == NT - 1
                            if final and ts == 3:
                                # very last tile: halve the norm+DMA pieces so
                                # the tail drains in parallel
                                nc.vector.tensor_scalar_mul(
                                    ostg[:, ts, 0:152], o[:, 0:152], rec
                                )
                                nc.vector.tensor_scalar_mul(
                                    ostg[:, ts, 152:D], o[:, 152:D], rec
                                )
                            else:
                                nc.vector.tensor_scalar_mul(
                                    ostg[:, ts, :], o[:, 0:D], rec
                                )
                        if lb == NB - 1 and a == 1 and tb == NT - 1:
                            # tail: spread issue cost over both hardware DGEs
                            nc.sync.dma_start(out=out_h[a][lb, tb, :, 0], in_=ostg[:, 0])
                            nc.scalar.dma_start(out=out_h[a][lb, tb, :, 1], in_=ostg[:, 1])
                            nc.sync.dma_start(out=out_h[a][lb, tb, :, 2], in_=ostg[:, 2])
                            nc.scalar.dma_start(
                                out=out_h[a][lb, tb, :, 3, 0:152], in_=ostg[:, 3, 0:152]
                            )
                            nc.sync.dma_start(
                                out=out_h[a][lb, tb, :, 3, 152:D], in_=ostg[:, 3, 152:D]
                            )
                        else:
                            nc.sync.dma_start(out=out_h[a][lb, tb], in_=ostg)

                for tb in range(NT):
                    ts_t = slice(tb * 512, (tb + 1) * 512)
                    # --- mm2 + exp for both attentions interleaved.
                    EA = ep.tile([128, NS, 512], bf16, tag="EA", name=f"EA{lb}{tb}")
                    EB = ep.tile([128, NS, 512], bf16, tag="EB", name=f"EB{lb}{tb}")
                    for st in range(NS):
                        ss = slice(st * 128, (st + 1) * 128)
                        scA = ps_sc.tile(
                            [128, 512], f32, tag="sc", name=f"scA{lb}{tb}{st}"
                        )
                        scB = ps_sc.tile(
                            [128, 512], f32, tag="sc", name=f"scB{lb}{tb}{st}"
                        )
                        for ke in range(2):
                            nc.tensor.matmul(
                                scA, ptA[:, ke, ss], a1[:, ke, ts_t],
                                start=(ke == 0), stop=False,
                            )
                        nc.tensor.matmul(
                            scA, ptAB2[:, ss], a1[:, 2, ts_t],
                            start=False, stop=True,
                        )
                        for ke in range(2):
                            nc.tensor.matmul(
                                scB, ptB[:, ke, ss], a1[:, ke, ts_t],
                                start=(ke == 0), stop=False,
                            )
                        nc.tensor.matmul(
                            scB, ptAB2[:, ss], a1[:, 3, ts_t],
                            start=False, stop=True,
                        )
                        nc.scalar.activation(out=EA[:, st, :], in_=scA, func=Exp)
                        nc.scalar.activation(out=EB[:, st, :], in_=scB, func=Exp)

                    if pending is not None:
                        mm4_block(*pending)
                    pending = (tb, EA, EB)
                mm4_block(*pending)
    _split_multi_waits(nc)
    return nc


def kernel(input1, input2, W2, b2, W3, b3, mode=None, **_ignored):
    from concourse.bass_utils import run_bass_kernel_spmd

    input1 = np.asarray(input1, dtype=np.float32)
    input2 = np.asarray(input2, dtype=np.float32)
    W2 = np.asarray(W2, dtype=np.float32)
    W3 = np.asarray(W3, dtype=np.float32)
    # bias b2/b3 add a per-(b,t) constant to the softmax logits — no effect.

    if "nc" not in _CACHE:
        _CACHE["nc"] = _build_nc()
    nc = _CACHE["nc"]

    import ml_dtypes

    # in1t: [B, 4, 128, T] fp16
    in1p = np.zeros((B, 384, T), np.float32)
    in1p[:, :D, :] = input1.transpose(0, 2, 1)
    in1t = np.zeros((B, 4, 128, T), np.float16)
    in1t[:, 0] = in1p[:, 0:128]
    in1t[:, 1] = in1p[:, 128:256]
    in1t[:, 2, 0:64] = in1p[:, 256:320]      # chunk2 data @ low half
    in1t[:, 3, 64:128] = in1p[:, 256:320]    # chunk2 data @ high half

    # in2t: [B, 2(h), 3(kd), 128, 512] fp16 (kd2 rows 256-383, 44 real)
    in2p = np.zeros((B, 384, S), np.float32)
    in2p[:, :D, :] = input2.transpose(0, 2, 1)
    in2t = np.ascontiguousarray(
        in2p.reshape(B, 3, 128, 2, 512).transpose(0, 3, 1, 2, 4)
    ).astype(np.float16)

    in2n = np.zeros((B, S, NA), np.float32)
    in2n[:, :, :D] = input2
    in2n[:, :, 300] = 1.0
    in2n = np.ascontiguousarray(
        in2n.reshape(B, S // 128, 128, NA).transpose(0, 2, 1, 3)
    ).astype(ml_dtypes.bfloat16)

    # packed weights [128, 15, 128] fp16; Wt = W.T padded, [d, e]
    WtA = np.zeros((384, 384), np.float32)
    WtA[:D, :D] = W2.T
    WtB = np.zeros((384, 384), np.float32)
    WtB[:D, :D] = W3.T
    wts = np.zeros((128, 15, 128), np.float16)
    for kd in range(3):
        r = slice(kd * 128, (kd + 1) * 128)
        wts[:, 0 + kd, :] = WtA[r, 0:128]
        wts[:, 3 + kd, :] = WtB[r, 0:128]
        wts[:, 6 + kd, :] = WtA[r, 128:256]
        wts[:, 9 + kd, :] = WtB[r, 128:256]
        wts[:, 12 + kd, 0:64] = WtA[r, 256:320]
        wts[:, 12 + kd, 64:128] = WtB[r, 256:320]

    in_maps = [
        {
            "wts": wts,
            "in1t": np.ascontiguousarray(in1t[c * NB : (c + 1) * NB]),
            "in2t": np.ascontiguousarray(in2t[c * NB : (c + 1) * NB]),
            "in2n": np.ascontiguousarray(in2n[c * NB : (c + 1) * NB]),
        }
        for c in range(NCORES)
    ]

    trace = bool(int(os.environ.get("KERNEL_PROFILE", "0")))
    if trace:
        _install_profile_hook()
    res = run_bass_kernel_spmd(nc, in_maps, list(range(NCORES)), trace=trace)
    _CACHE["last_exec_time_ns"] = res.exec_time_ns
    _CACHE["last_results"] = res

    def unswizzle(name):
        arr = np.concatenate([res.results[c][name] for c in range(NCORES)], axis=0)
        # [B, T//512, 128(p), 4(ts), D] -> [B, T, D] with t = tb*512 + ts*128 + p
        return np.ascontiguousarray(
            arr.transpose(0, 1, 3, 2, 4).reshape(B, T, D)
        )

    return unswizzle("out_a"), unswizzle("out_b")


# revision 30
# speedup vs baseline: 1.0253x; 1.0253x over previous
"""BiAttention (mode==1) Trainium2 Bass kernel.

Reference computation (per batch b, for (W,bias) in [(W2,b2),(W3,b3)]):
    proj   = input2[b] @ W.T + bias          # [S, D]
    scores = input1[b] @ proj.T              # [T, S]
    w      = softmax(scores, axis=-1)
    out    = w @ input2[b]                   # [T, D]
with B=16, T=2048, S=1024, D=300.

Key restructurings (validated vs reference in fp64/fp32, absmax-rel ~6e-3):
  * The bias contributes sum_e bias[e]*input1[b,t,e] to scores — constant in s,
    so it cancels in softmax and is dropped entirely.
  * Everything is computed in the transposed "scoresT" orientation [s, t] so
    that every matmul contracts over the partition dim with NO on-chip
    transposes:
        projT  [e, s] = Wt.T @ input2T      (lhsT = W.T chunks, rhs = input2T)
        scoresT[s, t] = projT.T @ input1T   (lhsT = projT slices, rhs = input1T)
        E = exp(scoresT)                    (no max-subtraction: |scores| < ~60,
                                             fine in f32 PSUM -> bf16)
        out[t, :304]  = E.T @ [input2 | 1]  (ones column accumulates the
                                             softmax denominator for free)
        out[t, d] = out[t, d] / out[t, 300]
  * proj/input1/W run in float16 (11-bit mantissa: scores err ~3e-3 absolute,
    harmless through exp; bf16 would NOT be - measured 3.9e-2 rel err).  E and
    input2 stay bf16 for mm4 (E spans e^+-60: needs bf16 range).  fp16 also
    halves the input DMA and gets LDWEIGHTS fast-weight-load (f32r cannot).
  * D=300 is chunked 128+128+128(44 real+zeros).  The two attentions share
    every loop; the ragged third chunk of mm1 packs both attentions' 64
    output columns into single 128-col weights ("pk"), producing projT-chunk2
    as [A @ partitions 0-63 | B @ partitions 64-127].  mm2's third chunk then
    uses that packed tile as lhsT for BOTH attentions, with two host-prepared
    rhs variants of input1T-chunk2 (data in the low / high partition half,
    zeros elsewhere) selecting the right half of the contraction.
    All matmuls stay uniform [K=128, M=128, N=512] - measured on HW that
    row-tiled 64-row pairs do NOT stream concurrently for this path and also
    break LDWEIGHTS pull-ahead on the following matmul.
  * PE warm-up: 8 dummy matmuls on a zeroed tile run during the initial DMA
    wait so the HAM clock gate reaches 2.4 GHz before real work (and no
    PE gaps afterwards, so it never re-throttles - worth ~2x on every
    matmul that would otherwise run cold).
  * mm4 (+softmax normalize) for t-block tb is issued AFTER mm2 of tb+1, so
    the scalar engine's exp stream (1.38us/st-group vs PE's ~1.35) never
    gates the PE within an mm2 phase.
  * Head DMAs are need-ordered and split to ~128-256KB so several of the 16
    DMA queues run in parallel (one queue sustains only ~96GB/s); issued on
    the two hardware DGE paths (sync + scalar).  gpsimd's software DGE is
    slow - not used.
  * Data-parallel over batch: 8 cores x 2 batches each, params replicated.
"""

import os

import numpy as np

B, T, S, D = 16, 2048, 1024, 300
NA = 304          # input2 free dim: 300 data + ones col at 300 + pad
                  # (301 fails walrus "ISA check" on the matmul)
NB = 2            # batches per core
NCORES = 8
NT = T // 512     # 4 t-blocks of 512
NS = S // 128     # 8 s-chunks of 128

_CACHE = {}


def _split_multi_waits(nc, maxw=1):
    """This walrus/CoreV3 build accepts at most one semaphore sync-wait per
    instruction ("Too many sync wait commands").  Tile attaches several to
    matmuls/DMAs/the tail Drain.  Post-scheduling, splice NOP carrier
    instructions (one wait each) in front of any instruction with more."""
    import concourse.mybir as mybir

    ctr = 0
    for fn in nc.m.functions:
        for blk in fn.blocks:
            insts = blk.instructions
            i = 0
            while i < len(insts):
                inst = insts[i]
                si = getattr(inst, "sync_info", None)
                waits = list(si.on_wait) if si is not None and si.on_wait else []
                if len(waits) > maxw:
                    si.on_wait = waits[len(waits) - maxw :]
                    carriers = []
                    for w in waits[: len(waits) - maxw]:
                        ctr += 1
                        carriers.append(
                            mybir.InstNoOp(
                                name=f"waitsplit-{ctr}",
                                engine=inst.engine,
                                ins=[],
                                outs=[],
                                sync_info=mybir.SyncInfo(on_wait=[w], on_update=[]),
                                bass_nofuse=True,
                            )
                        )
                    insts[i:i] = carriers
                    i += len(carriers)
                i += 1


def _install_profile_hook():
    """Synthesize the missing ``antenv.axon_hooks`` glue so run_bass_kernel_spmd
    trace=True can drive NTFF profiling through the injected libaxon_pjrt.so,
    and stub out the artifact upload (no bucket access here)."""
    import sys
    import types

    if "antenv.axon_hooks" not in sys.modules:
        mod = types.ModuleType("antenv.axon_hooks")
        holder = {}
        mod.set_axon_ntff_profile_hook = lambda h: holder.__setitem__("h", h)
        mod.get_axon_ntff_profile_hook = lambda: holder.get("h")
        sys.modules["antenv.axon_hooks"] = mod
        try:
            from trn_agent_boot.trn_boot import _ntff_profile_via_ctypes

            mod.set_axon_ntff_profile_hook(
                _ntff_profile_via_ctypes("/opt/axon/libaxon_pjrt.so")
            )
        except Exception:
            pass

    import concourse.bass_utils as bu

    if not getattr(bu, "_upload_stubbed", False):
        bu.upload_artifacts = lambda tmpdir: f"local:{tmpdir}"
        bu._upload_stubbed = True


def _build_nc():
    import concourse.bass as bass
    import concourse.mybir as mybir
    from concourse.tile import TileContext

    f32 = mybir.dt.float32
    f16 = mybir.dt.float16
    bf16 = mybir.dt.bfloat16
    Exp = mybir.ActivationFunctionType.Exp

    nc = bass.Bass("TRN2", target_bir_lowering=False, debug=False)
    # wts j-index (need-ordered for mm1's group sequence):
    #   A-ke0: j0-2 (kd0,1,2)   B-ke0: j3-5   A-ke1: j6-8   B-ke1: j9-11
    #   pk (A|B e2-cols packed): j12-14
    wts = nc.declare_dram_parameter("wts", [128, 15, 128], f16, isOutput=False)
    # in1t c-index: 0,1 = e-chunks 0,1; 2 = chunk2 data @ p0-63 (zeros @ hi);
    # 3 = chunk2 data @ p64-127 (zeros @ lo)
    in1t = nc.declare_dram_parameter("in1t", [NB, 4, 128, T], f16, isOutput=False)
    in2t = nc.declare_dram_parameter("in2t", [NB, 2, 3, 128, 512], f16, isOutput=False)
    in2n = nc.declare_dram_parameter("in2n", [NB, 128, NS, NA], bf16, isOutput=False)
    out_h = [
        nc.declare_dram_parameter("out_a", [NB, NT, 128, 4, D], f32, isOutput=True),
        nc.declare_dram_parameter("out_b", [NB, NT, 128, 4, D], f32, isOutput=True),
    ]

    with TileContext(nc) as tc:
        with (
            tc.tile_pool(name="warmp", bufs=1) as warmp,
            tc.tile_pool(name="wpool", bufs=1) as wpool,
            tc.tile_pool(name="a1p", bufs=2) as a1p,
            tc.tile_pool(name="a2p", bufs=2) as a2p,
            tc.tile_pool(name="a2np", bufs=2) as a2np,
            tc.tile_pool(name="projp", bufs=2) as projp,
            tc.tile_pool(name="ep", bufs=2) as ep,
            tc.tile_pool(name="outp", bufs=3) as outp,
            tc.tile_pool(name="recp", bufs=4) as recp,
            tc.tile_pool(name="ps_pj", bufs=2, space="PSUM") as ps_pj,
            tc.tile_pool(name="ps_sc", bufs=4, space="PSUM") as ps_sc,
            tc.tile_pool(name="ps_o", bufs=2, space="PSUM") as ps_o,
        ):
            # --- PE warm-up + gap fillers.  The head phase is DMA-paced: data
            # trickles in slower than the PE consumes it, and any PE idle
            # keeps the HAM clock gate at 1.2 GHz (everything then runs at
            # half clock until ~3.4us of sustained busy).  Dummy matmuls on an
            # intentionally-uninitialized tile (results never read, no deps,
            # so they slot into exactly the cycles the real stream would
            # stall) keep the busy-streak alive.  13 up front bridge preamble
            # -> first data; small fillers between the DMA-paced mm1 groups
            # bridge the trickle gaps.
            wsb = warmp.tile([128, 512], bf16)
            wscr = warmp.tile([128, 1], f32)
            nc.vector.memset(wsb, 0.0)
            wps = ps_o.tile([128, 512], f32, tag="o")

            def filler(n):
                for _ in range(n):
                    nc.tensor.matmul(
                        wps[:, 0:256], wsb[:, 0:128], wsb[:, 0:256],
                        start=True, stop=True,
                    )

            filler(15)

            # --- weights: resident all kernel, need-ordered DMAs.
            wt = wpool.tile([128, 15, 128], f16)

            for lb in range(NB):
                a2 = a2p.tile([128, 2, 3, 512], f16)
                a1 = a1p.tile([128, 4, T], f16)
                a2n = a2np.tile([128, NS, NA], bf16)
                if lb == 0:
                    # Both hardware DGEs (sync=SP, scalar=ACT) interleave the
                    # mm1 critical path: each dma_start costs ~650ns of issue
                    # time on its engine, so one engine alone delivers too
                    # late.  The dummy activation early on the scalar queue
                    # pre-triggers the ~2.7us exp ACT_TABLE_LOAD, which would
                    # otherwise stall the first real softmax by that much.
                    nc.sync.dma_start(out=wt[:, 0:3, :], in_=wts[:, 0:3, :])
                    nc.scalar.dma_start(out=a2[:, 0, 1], in_=in2t[lb, 0, 1])
                    nc.sync.dma_start(out=a2[:, 0, 0], in_=in2t[lb, 0, 0])
                    nc.scalar.dma_start(out=a2[:, 0, 2], in_=in2t[lb, 0, 2])
                    nc.scalar.activation(out=wscr, in_=wsb[:, 0:1], func=Exp)
                    nc.sync.dma_start(out=wt[:, 3:6, :], in_=wts[:, 3:6, :])
                    nc.scalar.dma_start(out=a2[:, 1, 1], in_=in2t[lb, 1, 1])
                    nc.sync.dma_start(out=a2[:, 1, 0], in_=in2t[lb, 1, 0])
                    nc.scalar.dma_start(out=a2[:, 1, 2], in_=in2t[lb, 1, 2])
                    nc.sync.dma_start(out=wt[:, 12:15, :], in_=wts[:, 12:15, :])
                    nc.sync.dma_start(out=wt[:, 6:12, :], in_=wts[:, 6:12, :])
                    for c in range(4):
                        nc.scalar.dma_start(
                            out=a1[:, c, 0:1024], in_=in1t[lb, c][:, 0:1024]
                        )
                    nc.sync.dma_start(out=a2n, in_=in2n[lb])
                    for c in range(4):
                        nc.sync.dma_start(
                            out=a1[:, c, 1024:T], in_=in1t[lb, c][:, 1024:T]
                        )
                else:
                    for h in range(2):
                        for kd in range(3):
                            nc.sync.dma_start(out=a2[:, h, kd], in_=in2t[lb, h, kd])
                    for c in range(4):
                        nc.sync.dma_start(out=a1[:, c, :], in_=in1t[lb, c])
                    nc.scalar.dma_start(out=a2n, in_=in2n[lb])

                # --- mm1: projT.  5 uniform 3-matmul groups per h:
                # A-ke0, B-ke0, A-ke1, B-ke1, pk (both attns' e2 cols packed).
                ptA = projp.tile([128, 2, S], f16)
                ptB = projp.tile([128, 2, S], f16)
                ptAB2 = projp.tile([128, S], f16)
                for h in range(2):
                    hs = slice(h * 512, (h + 1) * 512)
                    for g, dst in (
                        (0, lambda: ptA[:, 0, hs]),
                        (3, lambda: ptB[:, 0, hs]),
                        (12, lambda: ptAB2[:, hs]),
                        (6, lambda: ptA[:, 1, hs]),
                        (9, lambda: ptB[:, 1, hs]),
                    ):
                        pj = ps_pj.tile([128, 512], f32, tag="pj", name=f"pj{lb}{h}{g}")
                        for kd in range(3):
                            nc.tensor.matmul(
                                pj, wt[:, g + kd, :], a2[:, h, kd, :],
                                start=(kd == 0), stop=(kd == 2),
                            )
                        nc.vector.tensor_copy(dst(), pj)
                        if lb == 0 and h == 0:
                            filler(2)
                if lb == 0:
                    filler(6)

                # mm4 runs one t-block behind mm2 so the scalar engine's exp
                # stream (slower than the mm2 matmul stream) never gates PE.
                pending = None

                def mm4_block(tb, EA, EB):
                    for a in range(2):
                        E = EA if a == 0 else EB
                        ostg = outp.tile(
                            [128, 4, D], f32, tag="ostg", name=f"ostg{lb}{tb}{a}"
                        )
                        for ts in range(4):
                            o = ps_o.tile(
                                [128, NA], f32, tag="o", name=f"o{lb}{tb}{a}{ts}"
                            )
                            for st in range(NS):
                                nc.tensor.matmul(
                                    o,
                                    E[:, st, ts * 128 : (ts + 1) * 128],
                                    a2n[:, st, :],
                                    start=(st == 0),
                                    stop=(st == NS - 1),
                                )
                            rec = recp.tile(
                                [128, 1], f32, tag="rec", name=f"rec{lb}{tb}{a}{ts}"
                            )
                            nc.vector.reciprocal(rec, o[:, 300:301])
                            final = lb == NB - 1 and a == 1 and tb == NT - 1
                            if final and ts == 3:
                                # very last tile: halve the norm+DMA pieces so
                                # the tail drains in parallel
                                nc.vector.tensor_scalar_mul(
                                    ostg[:, ts, 0:152], o[:, 0:152], rec
                                )
                                nc.vector.tensor_scalar_mul(
                                    ostg[:, ts, 152:D], o[:, 152:D], rec
                                )
                            else:
                                nc.vector.tensor_scalar_mul(
                                    ostg[:, ts, :], o[:, 0:D], rec
                                )
                        if lb == NB - 1 and a == 1 and tb == NT - 1:
                            # tail: spread issue cost over both hardware DGEs
                            nc.sync.dma_start(out=out_h[a][lb, tb, :, 0], in_=ostg[:, 0])
                            nc.scalar.dma_start(out=out_h[a][lb, tb, :, 1], in_=ostg[:, 1])
                            nc.sync.dma_start(out=out_h[a][lb, tb, :, 2], in_=ostg[:, 2])
                            nc.scalar.dma_start(
                                out=out_h[a][lb, tb, :, 3, 0:152], in_=ostg[:, 3, 0:152]
                            )
                            nc.sync.dma_start(
                                out=out_h[a][lb, tb, :, 3, 152:D], in_=ostg[:, 3, 152:D]
                            )
                        else:
                            nc.sync.dma_start(out=out_h[a][lb, tb], in_=ostg)

                for tb in range(NT):
                    ts_t = slice(tb * 512, (tb + 1) * 512)
                    # --- mm2 + exp for both attentions interleaved.
                    EA = ep.tile([128, NS, 512], bf16, tag="EA", name=f"EA{lb}{tb}")
                    EB = ep.tile([128, NS, 512], bf16, tag="EB", name=f"EB{lb}{tb}")
                    for st in range(NS):
                        ss = slice(st * 128, (st + 1) * 128)
                        scA = ps_sc.tile(
                            [128, 512], f32, tag="sc", name=f"scA{lb}{tb}{st}"
                        )
                        scB = ps_sc.tile(
                            [128, 512], f32, tag="sc", name=f"scB{lb}{tb}{st}"
                        )
                        for ke in range(2):
                            nc.tensor.matmul(
                                scA, ptA[:, ke, ss], a1[:, ke, ts_t],
                                start=(ke == 0), stop=False,
                            )
                        nc.tensor.matmul(
                            scA, ptAB2[:, ss], a1[:, 2, ts_t],
                            start=False, stop=True,
                        )
                        for ke in range(2):
                            nc.tensor.matmul(
                                scB, ptB[:, ke, ss], a1[:, ke, ts_t],
                                start=(ke == 0), stop=False,
                            )
                        nc.tensor.matmul(
                            scB, ptAB2[:, ss], a1[:, 3, ts_t],
                            start=False, stop=True,
                        )
                        nc.scalar.activation(out=EA[:, st, :], in_=scA, func=Exp)
                        nc.scalar.activation(out=EB[:, st, :], in_=scB, func=Exp)

                    if pending is not None:
                        mm4_block(*pending)
                    pending = (tb, EA, EB)
                mm4_block(*pending)
    _split_multi_waits(nc)
    return nc


def kernel(input1, input2, W2, b2, W3, b3, mode=None, **_ignored):
    from concourse.bass_utils import run_bass_kernel_spmd

    input1 = np.asarray(input1, dtype=np.float32)
    input2 = np.asarray(input2, dtype=np.float32)
    W2 = np.asarray(W2, dtype=np.float32)
    W3 = np.asarray(W3, dtype=np.float32)
    # bias b2/b3 add a per-(b,t) constant to the softmax logits — no effect.

    if "nc" not in _CACHE:
        _CACHE["nc"] = _build_nc()
    nc = _CACHE["nc"]

    import ml_dtypes

    # in1t: [B, 4, 128, T] fp16
    in1p = np.zeros((B, 384, T), np.float32)
    in1p[:, :D, :] = input1.transpose(0, 2, 1)
    in1t = np.zeros((B, 4, 128, T), np.float16)
    in1t[:, 0] = in1p[:, 0:128]
    in1t[:, 1] = in1p[:, 128:256]
    in1t[:, 2, 0:64] = in1p[:, 256:320]      # chunk2 data @ low half
    in1t[:, 3, 64:128] = in1p[:, 256:320]    # chunk2 data @ high half

    # in2t: [B, 2(h), 3(kd), 128, 512] fp16 (kd2 rows 256-383, 44 real)
    in2p = np.zeros((B, 384, S), np.float32)
    in2p[:, :D, :] = input2.transpose(0, 2, 1)
    in2t = np.ascontiguousarray(
        in2p.reshape(B, 3, 128, 2, 512).transpose(0, 3, 1, 2, 4)
    ).astype(np.float16)

    in2n = np.zeros((B, S, NA), np.float32)
    in2n[:, :, :D] = input2
    in2n[:, :, 300] = 1.0
    in2n = np.ascontiguousarray(
        in2n.reshape(B, S // 128, 128, NA).transpose(0, 2, 1, 3)
    ).astype(ml_dtypes.bfloat16)

    # packed weights [128, 15, 128] fp16; Wt = W.T padded, [d, e]
    WtA = np.zeros((384, 384), np.float32)
    WtA[:D, :D] = W2.T
    WtB = np.zeros((384, 384), np.float32)
    WtB[:D, :D] = W3.T
    wts = np.zeros((128, 15, 128), np.float16)
    for kd in range(3):
        r = slice(kd * 128, (kd + 1) * 128)
        wts[:, 0 + kd, :] = WtA[r, 0:128]
        wts[:, 3 + kd, :] = WtB[r, 0:128]
        wts[:, 6 + kd, :] = WtA[r, 128:256]
        wts[:, 9 + kd, :] = WtB[r, 128:256]
        wts[:, 12 + kd, 0:64] = WtA[r, 256:320]
        wts[:, 12 + kd, 64:128] = WtB[r, 256:320]

    in_maps = [
        {
            "wts": wts,
            "in1t": np.ascontiguousarray(in1t[c * NB : (c + 1) * NB]),
            "in2t": np.ascontiguousarray(in2t[c * NB : (c + 1) * NB]),
            "in2n": np.ascontiguousarray(in2n[c * NB : (c + 1) * NB]),
        }
        for c in range(NCORES)
    ]

    trace = bool(int(os.environ.get("KERNEL_PROFILE", "0")))
    if trace:
        _install_profile_hook()
    res = run_bass_kernel_spmd(nc, in_maps, list(range(NCORES)), trace=trace)
    _CACHE["last_exec_time_ns"] = res.exec_time_ns
    _CACHE["last_results"] = res

    def unswizzle(name):
        arr = np.concatenate([res.results[c][name] for c in range(NCORES)], axis=0)
        # [B, T//512, 128(p), 4(ts), D] -> [B, T, D] with t = tb*512 + ts*128 + p
        return np.ascontiguousarray(
            arr.transpose(0, 1, 3, 2, 4).reshape(B, T, D)
        )

    return unswizzle("out_a"), unswizzle("out_b")
